# revision 34
# baseline (speedup 1.0000x reference)
"""Trainium2 Bass kernel: Luong-style attention with predictive alignment.

Math (see reference):
    h_t    = x[:, -1, :]                                   [B, H]
    t      = tanh(h_t @ W_p);  aligned = S*sigmoid(t @ v_p)
    scores[b,s] = sum_h x[b,s,h] * u[b,h],  u[b] = W_a @ h_t[b]
        (algebraic rewrite of (x @ W_a) . h_t -- avoids the B*S*H*H einsum)
    attn   = softmax(scores) * exp(-(pos-aligned)^2 / sigma2)
    ctx[b] = sum_s attn[b,s] * x[b,s,:]
    out    = tanh(concat(ctx, h_t) @ W_v)

Sharding: data-parallel over batch. 8 cores x 4 batches each; weights
replicated per core.

Schedule (single priority-ordered DMA ring for all bulk traffic):
  W_aT | x b0 c0-3 | W_p | x b0 c4-7 | x b1 | W_v[H:] | x b2 | x b3 | W_v[:H]
W_v rows [H:2H] reuse W_aT's SBUF (after the last u-broadcast reads it) and
W_v rows [0:H] reuse W_p's; the W_v[:H] half streams after the last x tile so
x never waits, and its matmuls are the last accumulation group anyway.

Per-batch chunk-granular softmax: the bias m1 = max(scores[cols 0:8]) is
fixed after chunk 3; every later chunk's exp + 2 context matmuls fire as
soon as its score STT lands (with the next chunk's STTs emitted ahead of
any scalar-dependent DVE op, so the DVE FIFO never stalls on them).  After
the last x tile lands only chunk 7's sliver remains: 2 score STTs, one
small exp, 4 context matmuls, and the transpose/merge.  Exact softmax:
e^{s-g2-m1}/sum(e^{s-m1}) == softmax(s)*gauss for any bias m1 (worst
|s - m1| on this input distribution is ~38, far below fp32 exp overflow at
~88).  The gauss window is applied as a precomputed table G = e^{-g2}
multiplying e^{s-m1}, and the 1/Z normalization is folded into the single
strided DVE write that scatters the transposed context into combT's
per-batch columns.  sigmoid(z) is computed as 1/(1+e^{-z}) so the scalar
engine never swaps activation tables mid-kernel (Sigmoid lives in a
different table set than Exp/Square/Tanh; each swap costs ~1.5us).
"""

import math
from contextlib import ExitStack

import numpy as np

import concourse.bass as bass
import concourse.bass_isa as bass_isa
import concourse.mybir as mybir
import concourse.tile as tile
from concourse import bacc
from concourse.bass_utils import run_bass_kernel_spmd

B, S, H, SIZE = 32, 2048, 1024, 1024
NCORES = 8
BPC = B // NCORES          # batches per core
NCH = 8                    # x chunks per batch
SCH = S // NCH             # 256 sequence positions per chunk
A = 2                      # sub-slices (128 s-positions each) per chunk
COLS = NCH * A             # 16 score columns per batch
NCOLS_A = 14               # phase-A columns (chunks 0-6)
F32 = mybir.dt.float32
F32R = mybir.dt.float32r
SIGMA_SQ = 2.0 * (S / 2.0 / 2.0) ** 2    # D = S//2; 2*(D/2)^2 = 524288
INV_SG = 1.0 / math.sqrt(SIGMA_SQ)

_CACHE = {}
TRACE = False


def _build():
    AF = mybir.ActivationFunctionType
    OP = mybir.AluOpType
    nc = bacc.Bacc()

    x_s = nc.dram_tensor("x_s", [BPC, S, H], F32, kind="ExternalInput")
    w_p = nc.dram_tensor("w_p", [H, H], F32, kind="ExternalInput")
    w_at = nc.dram_tensor("w_at", [H, H], F32, kind="ExternalInput")
    w_v = nc.dram_tensor("w_v", [2 * H, SIZE], F32, kind="ExternalInput")
    htk = nc.dram_tensor("htk", [128, 8 * BPC], F32, kind="ExternalInput")
    vrep = nc.dram_tensor("vrep", [BPC, H], F32, kind="ExternalInput")
    posd = nc.dram_tensor("pos", [128, COLS], F32, kind="ExternalInput")
    idd = nc.dram_tensor("ident", [128, 128], F32, kind="ExternalInput")
    outd = nc.dram_tensor("out", [BPC, SIZE], F32, kind="ExternalOutput")

    with tile.TileContext(nc) as tc, ExitStack() as ctx:
        const = ctx.enter_context(tc.tile_pool(name="const", bufs=1))
        wts = ctx.enter_context(tc.tile_pool(name="wts", bufs=1))
        xs = ctx.enter_context(tc.tile_pool(name="xs", bufs=12))
        ubcp = ctx.enter_context(tc.tile_pool(name="ubcp", bufs=4))
        ctxp = ctx.enter_context(tc.tile_pool(name="ctxp", bufs=2))
        prodp = ctx.enter_context(tc.tile_pool(name="prodp", bufs=2))
        small = ctx.enter_context(tc.tile_pool(name="small", bufs=2))
        psUbc = ctx.enter_context(
            tc.tile_pool(name="psUbc", bufs=1, space=bass.MemorySpace.PSUM)
        )
        psCtx = ctx.enter_context(
            tc.tile_pool(name="psCtx", bufs=1, space=bass.MemorySpace.PSUM)
        )
        psT = ctx.enter_context(
            tc.tile_pool(name="psT", bufs=1, space=bass.MemorySpace.PSUM)
        )
        psO = ctx.enter_context(
            tc.tile_pool(name="psO", bufs=1, space=bass.MemorySpace.PSUM)
        )
        dpool = ctx.enter_context(
            tc.tile_pool(name="dram", bufs=1, space=bass.MemorySpace.DRAM)
        )

        # ---- small inputs ride the gpsimd ring; bulk traffic owns sync ----
        combT = const.tile([128, 8 * BPC * 2], F32R)  # combined^T: [p, 4k+b]
        v_sb = const.tile([BPC, H], F32)
        pos_sb = const.tile([128, COLS], F32)
        id_sb = const.tile([128, 128], F32)
        tta = const.tile([BPC, H], F32)
        alb = const.tile([BPC, 1], F32)
        out_sb = const.tile([BPC, SIZE], F32)

        nc.gpsimd.dma_start(out=combT[:, 32:64], in_=htk[:, :].bitcast(F32R))
        nc.gpsimd.dma_start(out=v_sb, in_=vrep[:, :])
        nc.gpsimd.dma_start(out=pos_sb, in_=posd[:, :])
        nc.gpsimd.dma_start(out=id_sb, in_=idd[:, :])

        # ---- bulk stream alternates between the sync and scalar queues so
        # one queue's descriptor-feed gaps are covered by the other; both
        # queues keep the same priority order: W_aT first (u gates scores)
        wa_sb = wts.tile([128, 8, H], F32R, tag="w1")
        nc.sync.dma_start(
            out=wa_sb[:, 0:4, :],
            in_=w_at[0 : H // 2, :].rearrange("(k p) j -> p k j", p=128).bitcast(F32R),
        )
        nc.sync.dma_start(
            out=wa_sb[:, 4:8, :],
            in_=w_at[H // 2 :, :].rearrange("(k p) j -> p k j", p=128).bitcast(F32R),
        )

        # u[b] broadcast across partitions, computed directly on PE: lhsT is
        # the h_t column replicated along its free dim (step-0 AP), so
        # out[p, h] = sum_k h_t[b,k] W_aT[k,h] = u[b,h] for every partition p.
        ubc_tiles = [None] * BPC

        def emit_ubc(b):
            ub_ps = psUbc.tile([128, H], F32, tag="ub", name=f"ubps_{b}")
            for k in range(8):
                c0 = combT[:, 32 + 4 * k + b : 32 + 4 * k + b + 1]
                lhs = bass.AP(
                    tensor=c0.tensor, offset=c0.offset, ap=[c0.ap[0], [0, 128]]
                )
                for h2 in range(2):
                    nc.tensor.matmul(
                        ub_ps[:, 512 * h2 : 512 * (h2 + 1)],
                        lhs,
                        wa_sb[:, k, 512 * h2 : 512 * (h2 + 1)],
                        start=(k == 0),
                        stop=(k == 7),
                    )
            ubc = ubcp.tile([128, H], F32, tag="ubc", name=f"ubc_{b}")
            nc.scalar.copy(ubc, ub_ps)
            ubc_tiles[b] = ubc

        emit_ubc(0)
        emit_ubc(1)
        emit_ubc(2)
        emit_ubc(3)

        # ---- x DMAs (sync ring) ----
        all_x = [[None] * NCH for _ in range(BPC)]

        def emit_x_dmas(b, cs):
            for c in cs:
                xt = xs.tile([128, A, H], F32R, tag="xt", name=f"xt_{b}_{c}")
                nc.sync.dma_start(
                    out=xt,
                    in_=x_s[b, c * SCH : (c + 1) * SCH, :]
                    .rearrange("(p a) h -> p a h", p=128)
                    .bitcast(F32R),
                )
                all_x[b][c] = xt

        emit_x_dmas(0, range(4))

        # ---- W_p + t/aligned (f32r matmul; lhsT = combT h_t cols) ----
        wp_sb = wts.tile([128, 8, H], F32R, tag="w0")
        nc.sync.dma_start(
            out=wp_sb, in_=w_p[:, :].rearrange("(k p) j -> p k j", p=128).bitcast(F32R)
        )
        emit_x_dmas(0, range(4, NCH))
        ab_d = dpool.tile([BPC, 1], F32)
        ab_tiles = [
            const.tile([128, 1], F32, name=f"abb_{bb}") for bb in range(BPC)
        ]

        def emit_aligned_section():
            # t = tanh(h_t @ W_p); aligned = S*sigmoid(t @ v_p)
            ps_t = psO.tile([BPC, H], F32, tag="po")
            for k in range(8):
                lhs = combT[:, 32 + 4 * k : 32 + 4 * k + 4]
                for h2 in range(2):
                    nc.tensor.matmul(
                        ps_t[:, 512 * h2 : 512 * (h2 + 1)],
                        lhs,
                        wp_sb[:, k, 512 * h2 : 512 * (h2 + 1)],
                        start=(k == 0),
                        stop=(k == 7),
                    )
            nc.scalar.activation(out=tta, in_=ps_t, func=AF.Tanh)

            prod0 = prodp.tile([BPC, H], F32, tag="p0")
            al_r = small.tile([BPC, 1], F32, tag="alr")
            nc.vector.scalar_tensor_tensor(
                out=prod0,
                in0=tta,
                scalar=1.0,
                in1=v_sb,
                op0=OP.mult,
                op1=OP.mult,
                accum_out=al_r,
            )
            # sigmoid via resident Exp table (a Sigmoid activation would
            # force two ACT_TABLE_LOAD swaps on the scalar engine mid-kernel)
            e_neg = small.tile([BPC, 1], F32, tag="eneg")
            nc.scalar.activation(out=e_neg, in_=al_r, func=AF.Exp, bias=0.0, scale=-1.0)
            e_p1 = small.tile([BPC, 1], F32, tag="ep1")
            nc.vector.tensor_scalar_add(e_p1, e_neg, 1.0)
            sigv = small.tile([BPC, 1], F32, tag="sigv")
            nc.vector.reciprocal(sigv, e_p1)
            nc.scalar.mul(alb, sigv, -float(S) * INV_SG)  # alb = -aligned/sg
            # alignment roundtrip + per-batch broadcasts ride the otherwise
            # idle gpsimd ring so they never queue behind bulk traffic
            nc.gpsimd.dma_start(out=ab_d[:, :], in_=alb)
            for bb in range(BPC):
                nc.gpsimd.dma_start(
                    out=ab_tiles[bb], in_=ab_d[bb : bb + 1, :].to_broadcast((128, 1))
                )

        # ---- per-batch: scores + chunk-granular softmax/context ----
        # The bias m1 is fixed after chunk 3 (cols 0-7); every later chunk's
        # exp + 2 context matmuls run as soon as its score STT lands, so for
        # the last-streamed batch only chunk 7's sliver remains post-stream.
        # Exact softmax: e^{s-g2-m1}/sum(e^{s-m1}) == softmax(s)*gauss for
        # any bias m1 (worst |s-m1| here is far below fp32 exp overflow).
        NCH0 = 4          # chunks covered by the m1 bias phase
        C0 = NCH0 * A     # cols 0..7

        def batch_section(b, after_scores=None, mid_hook=None):
            ubc = ubc_tiles[b]
            sc_b = small.tile([128, COLS], F32, tag="scb", name=f"scb_{b}")

            def emit_stt(col):
                c, a = col // A, col % A
                prod = prodp.tile([128, H], F32, tag="p0", name=f"pr_{b}_{col}")
                nc.vector.scalar_tensor_tensor(
                    out=prod,
                    in0=all_x[b][c][:, a, :].bitcast(F32),
                    scalar=1.0,
                    in1=ubc,
                    op0=OP.mult,
                    op1=OP.mult,
                    accum_out=sc_b[:, col : col + 1],
                )

            for col in range(C0):
                emit_stt(col)

            # fix bias m1 = max over cols 0..7
            mx_p = small.tile([128, 1], F32, tag="mxp", name=f"mxp_{b}")
            nc.vector.reduce_max(
                out=mx_p, in_=sc_b[:, 0:C0], axis=mybir.AxisListType.X
            )
            mcast = small.tile([128, 1], F32, tag="mcast", name=f"mcast_{b}")
            nc.gpsimd.partition_all_reduce(
                mcast, mx_p, channels=128, reduce_op=bass_isa.ReduceOp.max
            )
            negm = small.tile([128, 1], F32, tag="negm", name=f"negm_{b}")
            nc.scalar.mul(negm, mcast, -1.0)

            # lookahead: chunk 4's score STTs go ahead of the g2-dependent
            # at0 multiply in the DVE queue (g2 can lag on batch 0)
            for col in range(C0, C0 + A):
                emit_stt(col)
            if after_scores is not None:
                after_scores()

            ew0 = small.tile([128, C0], F32, tag="ew", name=f"ew0_{b}")
            zp0 = small.tile([128, 1], F32, tag="zp0", name=f"zp0_{b}")
            nc.scalar.activation(
                out=ew0,
                in_=sc_b[:, 0:C0],
                func=AF.Exp,
                bias=negm,
                scale=1.0,
                accum_out=zp0,
            )

            g2 = small.tile([128, COLS], F32, tag="g2", name=f"g2_{b}")
            nc.scalar.activation(
                out=g2, in_=pos_sb, func=AF.Square, bias=ab_tiles[b], scale=INV_SG
            )
            # gauss factors G = e^{-g2}; at = e^{s-m1} * G replaces the
            # per-chunk (sub, exp) pair so nothing g2-dependent sits in the
            # DVE queue ahead of later score STTs
            gss = small.tile([128, COLS], F32, tag="gss", name=f"gss_{b}")
            nc.scalar.activation(out=gss, in_=g2, func=AF.Exp, bias=0.0, scale=-1.0)
            at0 = small.tile([128, C0], F32R, tag="at0", name=f"at0_{b}")
            nc.vector.tensor_mul(out=at0, in0=ew0, in1=gss[:, 0:C0])

            ps_c = psCtx.tile([1, H], F32, tag="pc", name=f"pc_{b}")
            for col in range(C0):
                c, a = col // A, col % A
                for h2 in range(2):
                    nc.tensor.matmul(
                        ps_c[:, 512 * h2 : 512 * (h2 + 1)],
                        at0[:, col : col + 1],
                        all_x[b][c][:, a, 512 * h2 : 512 * (h2 + 1)],
                        start=(col == 0),
                        stop=False,
                    )
            if mid_hook is not None:
                # PE work that's ready during this batch's softmax window
                mid_hook()

            # chunks 4..7: exp + context matmuls fire per chunk as it lands;
            # the NEXT chunk's STTs are emitted first (lookahead) so no
            # scalar-dependent DVE op ever blocks a later score STT
            zrun = zp0
            for ch in range(NCH0, NCH):
                col0 = ch * A
                if ch + 1 < NCH:
                    for col in range((ch + 1) * A, (ch + 2) * A):
                        emit_stt(col)
                ewc = small.tile([128, A], F32, tag=f"ew{ch}", name=f"ew{ch}_{b}")
                zpc = small.tile([128, 1], F32, tag=f"zp{ch}", name=f"zp{ch}_{b}")
                nc.scalar.activation(
                    out=ewc,
                    in_=sc_b[:, col0 : col0 + A],
                    func=AF.Exp,
                    bias=negm,
                    scale=1.0,
                    accum_out=zpc,
                )
                atc = small.tile([128, A], F32R, tag=f"at{ch}", name=f"at{ch}_{b}")
                nc.vector.tensor_mul(
                    out=atc, in0=ewc, in1=gss[:, col0 : col0 + A]
                )
                for col in range(col0, col0 + A):
                    c, a = col // A, col % A
                    for h2 in range(2):
                        nc.tensor.matmul(
                            ps_c[:, 512 * h2 : 512 * (h2 + 1)],
                            atc[:, col - col0 : col - col0 + 1],
                            all_x[b][c][:, a, 512 * h2 : 512 * (h2 + 1)],
                            start=False,
                            stop=(col == COLS - 1 and h2 == 1),
                        )
                zn = small.tile([128, 1], F32, tag=f"zr{ch}", name=f"zr{ch}_{b}")
                nc.vector.tensor_add(out=zn, in0=zrun, in1=zpc)
                zrun = zn

            zsum = small.tile([128, 1], F32, tag="zsum", name=f"zsum_{b}")
            nc.gpsimd.partition_all_reduce(
                zsum, zrun, channels=128, reduce_op=bass_isa.ReduceOp.add
            )
            zinv = small.tile([128, 1], F32, tag="zinv", name=f"zinv_{b}")
            nc.vector.reciprocal(zinv, zsum)

            # ctx out of PSUM (split across scalar+vector), transpose
            # 128-blocks, then one strided DVE write folds 1/Z and lands
            # all 8 combT columns for this batch
            ctx_t = ctxp.tile([1, H], F32, tag="ctx", name=f"ctx_{b}")
            nc.scalar.copy(ctx_t[0:1, 0 : H // 2], ps_c[0:1, 0 : H // 2])
            nc.vector.tensor_copy(out=ctx_t[0:1, H // 2 : H], in_=ps_c[0:1, H // 2 : H])
            ps_ct = psT.tile([128, 8], F32, tag="pt", name=f"pct_{b}")
            for k in range(8):
                nc.tensor.transpose(
                    ps_ct[:, k : k + 1],
                    ctx_t[0:1, 128 * k : 128 * (k + 1)],
                    id_sb[0:1, 0:1],
                )
            cT = combT[:, b : b + 1]
            comb_cols = bass.AP(
                tensor=cT.tensor, offset=cT.offset, ap=[cT.ap[0], [4, 8]]
            )
            nc.vector.tensor_scalar_mul(comb_cols, ps_ct, zinv)

        # final-output accumulator [BPC, SIZE]; h_t-half matmuls run early
        ps_o = psO.tile([BPC, SIZE], F32, tag="po")

        def emit_final_hhalf():
            for k in range(8, 16):
                lhs = combT[:, 4 * k : 4 * k + 4]
                for h2 in range(2):
                    nc.tensor.matmul(
                        ps_o[:, 512 * h2 : 512 * (h2 + 1)],
                        lhs,
                        wv1_sb[:, k % 8, 512 * h2 : 512 * (h2 + 1)],
                        start=(k == 8),
                        stop=False,
                    )

        batch_section(0, after_scores=emit_aligned_section)
        emit_x_dmas(1, range(NCH))

        # W_v rows [H:2H] reuse W_aT's SBUF slot; DMA waits on ubc_3's reads
        # (all ubc broadcasts are emitted upfront, so that's ~25us in)
        wv1_sb = wts.tile([128, 8, SIZE], F32R, tag="w1")
        nc.sync.dma_start(
            out=wv1_sb,
            in_=w_v[H : 2 * H, :].rearrange("(k p) o -> p k o", p=128).bitcast(F32R),
        )
        batch_section(1)
        emit_x_dmas(2, range(NCH))
        batch_section(2, mid_hook=emit_final_hhalf)
        emit_x_dmas(3, range(NCH))

        # W_v rows [0:H] reuse W_p's slot; queued after all x so the x
        # stream is never stalled behind it (its matmuls are last anyway)
        wv0_sb = wts.tile([128, 8, SIZE], F32R, tag="w0")
        for k in range(8):
            nc.sync.dma_start(
                out=wv0_sb[:, k : k + 1, :],
                in_=w_v[128 * k : 128 * (k + 1), :]
                .rearrange("(k p) o -> p k o", p=128)
                .bitcast(F32R),
            )
        batch_section(3)

        for k in range(8):
            lhs = combT[:, 4 * k : 4 * k + 4]
            for h2 in range(2):
                nc.tensor.matmul(
                    ps_o[:, 512 * h2 : 512 * (h2 + 1)],
                    lhs,
                    wv0_sb[:, k, 512 * h2 : 512 * (h2 + 1)],
                    start=False,
                    stop=(k == 7 and h2 == 1),
                )
        # tanh+store in quarters so each store overlaps the next tanh
        Q = SIZE // 4
        for q in range(4):
            nc.scalar.activation(
                out=out_sb[:, Q * q : Q * (q + 1)],
                in_=ps_o[:, Q * q : Q * (q + 1)],
                func=AF.Tanh,
            )
            ring = nc.gpsimd if q % 2 == 0 else nc.sync
            ring.dma_start(
                out=outd[:, Q * q : Q * (q + 1)], in_=out_sb[:, Q * q : Q * (q + 1)]
            )

    nc.compile()
    return nc


def _host_prep(x, W_p, v_p, W_a, W_v):
    x = np.ascontiguousarray(np.asarray(x, dtype=np.float32))
    W_p = np.ascontiguousarray(np.asarray(W_p, dtype=np.float32))
    v_p = np.asarray(v_p, dtype=np.float32).reshape(-1)
    W_aT = np.ascontiguousarray(np.asarray(W_a, dtype=np.float32).T)
    W_v = np.ascontiguousarray(np.asarray(W_v, dtype=np.float32))

    h_all = np.ascontiguousarray(x[:, -1, :])  # [B, H]
    vrep = np.ascontiguousarray(np.broadcast_to(v_p.reshape(1, H), (BPC, H)))
    cols = np.arange(COLS)
    p = np.arange(128)
    pos = ((cols[None, :] // A) * SCH + p[:, None] * A + (cols[None, :] % A)).astype(
        np.float32
    )
    pos = np.ascontiguousarray(pos)
    ident = np.eye(128, dtype=np.float32)

    in_maps = []
    for c in range(NCORES):
        hT = h_all[BPC * c : BPC * (c + 1)].T  # [H, BPC]
        htk_a = np.ascontiguousarray(
            hT.reshape(8, 128, BPC).transpose(1, 0, 2).reshape(128, 8 * BPC)
        )
        in_maps.append(
            dict(
                x_s=np.ascontiguousarray(x[BPC * c : BPC * (c + 1)]),
                w_p=W_p,
                w_at=W_aT,
                w_v=W_v,
                htk=htk_a,
                vrep=vrep,
                pos=pos,
                ident=ident,
            )
        )
    return in_maps


def kernel(x, W_p, v_p, W_a, W_v):
    if "nc" not in _CACHE:
        _CACHE["nc"] = _build()
    nc = _CACHE["nc"]
    in_maps = _host_prep(x, W_p, v_p, W_a, W_v)
    res = run_bass_kernel_spmd(nc, in_maps, core_ids=list(range(NCORES)), trace=TRACE)
    _CACHE["last_results"] = res
    return np.concatenate([r["out"] for r in res.results], axis=0)


# revision 36
# speedup vs baseline: 1.1511x; 1.1511x over previous
"""Trainium2 Bass kernel: Luong-style attention with predictive alignment.

Math (see reference):
    h_t    = x[:, -1, :]                                   [B, H]
    t      = tanh(h_t @ W_p);  aligned = S*sigmoid(t @ v_p)
    scores[b,s] = sum_h x[b,s,h] * u[b,h],  u[b] = W_a @ h_t[b]
        (algebraic rewrite of (x @ W_a) . h_t -- avoids the B*S*H*H einsum)
    attn   = softmax(scores) * exp(-(pos-aligned)^2 / sigma2)
    ctx[b] = sum_s attn[b,s] * x[b,s,:]
    out    = tanh(concat(ctx, h_t) @ W_v)

Sharding: data-parallel over batch. 8 cores x 4 batches each; weights
replicated per core.

Schedule (single priority-ordered DMA ring for all bulk traffic):
  W_aT | x b0 c0-3 | W_p | x b0 c4-7 | x b1 | W_v[H:] | x b2 | x b3 | W_v[:H]
W_v rows [H:2H] reuse W_aT's SBUF (after the last u-broadcast reads it) and
W_v rows [0:H] reuse W_p's; the W_v[:H] half streams after the last x tile so
x never waits, and its matmuls are the last accumulation group anyway.

Per-batch chunk-granular softmax: the bias m1 = max(scores[cols 0:8]) is
fixed after chunk 3; every later chunk's exp + 2 context matmuls fire as
soon as its score STT lands (with the next chunk's STTs emitted ahead of
any scalar-dependent DVE op, so the DVE FIFO never stalls on them).  After
the last x tile lands only chunk 7's sliver remains: 2 score STTs, one
small exp, 4 context matmuls, and the transpose/merge.  Exact softmax:
e^{s-g2-m1}/sum(e^{s-m1}) == softmax(s)*gauss for any bias m1 (worst
|s - m1| on this input distribution is ~38, far below fp32 exp overflow at
~88).  The gauss window is applied as a precomputed table G = e^{-g2}
multiplying e^{s-m1}, and the 1/Z normalization is folded into the single
strided DVE write that scatters the transposed context into combT's
per-batch columns.  sigmoid(z) is computed as 1/(1+e^{-z}) so the scalar
engine never swaps activation tables mid-kernel (Sigmoid lives in a
different table set than Exp/Square/Tanh; each swap costs ~1.5us).
"""

import math
from contextlib import ExitStack

import numpy as np

import concourse.bass as bass
import concourse.bass_isa as bass_isa
import concourse.mybir as mybir
import concourse.tile as tile
from concourse import bacc
from concourse.bass_utils import run_bass_kernel_spmd

B, S, H, SIZE = 32, 2048, 1024, 1024
NCORES = 8
BPC = B // NCORES          # batches per core
NCH = 8                    # x chunks per batch
SCH = S // NCH             # 256 sequence positions per chunk
A = 2                      # sub-slices (128 s-positions each) per chunk
COLS = NCH * A             # 16 score columns per batch
NCOLS_A = 14               # phase-A columns (chunks 0-6)
F32 = mybir.dt.float32
F32R = mybir.dt.float32r
F16 = mybir.dt.float16
SIGMA_SQ = 2.0 * (S / 2.0 / 2.0) ** 2    # D = S//2; 2*(D/2)^2 = 524288
INV_SG = 1.0 / math.sqrt(SIGMA_SQ)

_CACHE = {}
TRACE = False


def _build():
    AF = mybir.ActivationFunctionType
    OP = mybir.AluOpType
    nc = bacc.Bacc()

    x_s = nc.dram_tensor("x_s", [BPC, S, H], F16, kind="ExternalInput")
    w_p = nc.dram_tensor("w_p", [H, H], F16, kind="ExternalInput")
    w_at = nc.dram_tensor("w_at", [H, H], F16, kind="ExternalInput")
    w_v = nc.dram_tensor("w_v", [2 * H, SIZE], F16, kind="ExternalInput")
    htk = nc.dram_tensor("htk", [128, 8 * BPC], F16, kind="ExternalInput")
    vrep = nc.dram_tensor("vrep", [BPC, H], F32, kind="ExternalInput")
    posd = nc.dram_tensor("pos", [128, COLS], F32, kind="ExternalInput")
    idd = nc.dram_tensor("ident", [128, 128], F32, kind="ExternalInput")
    outd = nc.dram_tensor("out", [BPC, SIZE], F32, kind="ExternalOutput")

    with tile.TileContext(nc) as tc, ExitStack() as ctx:
        const = ctx.enter_context(tc.tile_pool(name="const", bufs=1))
        wts = ctx.enter_context(tc.tile_pool(name="wts", bufs=1))
        xs = ctx.enter_context(tc.tile_pool(name="xs", bufs=12))
        ubcp = ctx.enter_context(tc.tile_pool(name="ubcp", bufs=4))
        ctxp = ctx.enter_context(tc.tile_pool(name="ctxp", bufs=2))
        prodp = ctx.enter_context(tc.tile_pool(name="prodp", bufs=2))
        small = ctx.enter_context(tc.tile_pool(name="small", bufs=2))
        psUbc = ctx.enter_context(
            tc.tile_pool(name="psUbc", bufs=1, space=bass.MemorySpace.PSUM)
        )
        psCtx = ctx.enter_context(
            tc.tile_pool(name="psCtx", bufs=1, space=bass.MemorySpace.PSUM)
        )
        psT = ctx.enter_context(
            tc.tile_pool(name="psT", bufs=1, space=bass.MemorySpace.PSUM)
        )
        psO = ctx.enter_context(
            tc.tile_pool(name="psO", bufs=1, space=bass.MemorySpace.PSUM)
        )
        dpool = ctx.enter_context(
            tc.tile_pool(name="dram", bufs=1, space=bass.MemorySpace.DRAM)
        )

        # ---- small inputs ride the gpsimd ring; bulk traffic owns sync ----
        pad0 = const.tile([128, 320], F32)
        combT = const.tile([128, 8 * BPC * 2], F16)  # combined^T: [p, 4k+b]
        v_sb = const.tile([BPC, H], F32)
        pos_sb = const.tile([128, COLS], F32)
        id_sb = const.tile([128, 128], F32)
        tta = const.tile([BPC, H], F32)
        alb = const.tile([BPC, 1], F32)
        out_sb = const.tile([BPC, SIZE], F32)

        nc.gpsimd.dma_start(out=combT[:, 32:64], in_=htk[:, :])
        nc.gpsimd.dma_start(out=v_sb, in_=vrep[:, :])
        nc.gpsimd.dma_start(out=pos_sb, in_=posd[:, :])
        nc.gpsimd.dma_start(out=id_sb, in_=idd[:, :])

        # ---- bulk stream alternates between the sync and scalar queues so
        # one queue's descriptor-feed gaps are covered by the other; both
        # queues keep the same priority order: W_aT first (u gates scores)
        wa_sb = wts.tile([128, 8, H], F16, tag="w1")
        nc.sync.dma_start(
            out=wa_sb[:, 0:4, :],
            in_=w_at[0 : H // 2, :].rearrange("(k p) j -> p k j", p=128),
        )
        nc.sync.dma_start(
            out=wa_sb[:, 4:8, :],
            in_=w_at[H // 2 :, :].rearrange("(k p) j -> p k j", p=128),
        )

        # u[b] broadcast across partitions, computed directly on PE: lhsT is
        # the h_t column replicated along its free dim (step-0 AP), so
        # out[p, h] = sum_k h_t[b,k] W_aT[k,h] = u[b,h] for every partition p.
        ubc_tiles = [None] * BPC

        def emit_ubc(b):
            ub_ps = psUbc.tile([128, H], F32, tag="ub", name=f"ubps_{b}")
            for k in range(8):
                c0 = combT[:, 32 + 4 * k + b : 32 + 4 * k + b + 1]
                lhs = bass.AP(
                    tensor=c0.tensor, offset=c0.offset, ap=[c0.ap[0], [0, 128]]
                )
                for h2 in range(2):
                    nc.tensor.matmul(
                        ub_ps[:, 512 * h2 : 512 * (h2 + 1)],
                        lhs,
                        wa_sb[:, k, 512 * h2 : 512 * (h2 + 1)],
                        start=(k == 0),
                        stop=(k == 7),
                    )
            ubc = ubcp.tile([128, H], F16, tag="ubc", name=f"ubc_{b}")
            nc.scalar.copy(ubc, ub_ps)
            ubc_tiles[b] = ubc

        emit_ubc(0)
        emit_ubc(1)
        emit_ubc(2)
        emit_ubc(3)

        # ---- x DMAs (sync ring) ----
        all_x = [[None] * NCH for _ in range(BPC)]

        def emit_x_dmas(b, cs):
            for c in cs:
                xt = xs.tile([128, A, H], F16, tag="xt", name=f"xt_{b}_{c}")
                nc.sync.dma_start(
                    out=xt,
                    in_=x_s[b, c * SCH : (c + 1) * SCH, :]
                    .rearrange("(p a) h -> p a h", p=128),
                )
                all_x[b][c] = xt

        emit_x_dmas(0, range(4))

        # ---- W_p + t/aligned (f32r matmul; lhsT = combT h_t cols) ----
        wp_sb = wts.tile([128, 8, H], F16, tag="w0")
        nc.sync.dma_start(
            out=wp_sb, in_=w_p[:, :].rearrange("(k p) j -> p k j", p=128)
        )
        emit_x_dmas(0, range(4, NCH))
        ab_d = dpool.tile([BPC, 1], F32)
        ab_tiles = [
            const.tile([128, 1], F32, name=f"abb_{bb}") for bb in range(BPC)
        ]

        def emit_aligned_section():
            # t = tanh(h_t @ W_p); aligned = S*sigmoid(t @ v_p)
            ps_t = psO.tile([BPC, H], F32, tag="po")
            for k in range(8):
                lhs = combT[:, 32 + 4 * k : 32 + 4 * k + 4]
                for h2 in range(2):
                    nc.tensor.matmul(
                        ps_t[:, 512 * h2 : 512 * (h2 + 1)],
                        lhs,
                        wp_sb[:, k, 512 * h2 : 512 * (h2 + 1)],
                        start=(k == 0),
                        stop=(k == 7),
                    )
            nc.scalar.activation(out=tta, in_=ps_t, func=AF.Tanh)

            prod0 = prodp.tile([BPC, H], F32, tag="pal")
            al_r = small.tile([BPC, 1], F32, tag="alr")
            nc.vector.scalar_tensor_tensor(
                out=prod0,
                in0=tta,
                scalar=1.0,
                in1=v_sb,
                op0=OP.mult,
                op1=OP.mult,
                accum_out=al_r,
            )
            # sigmoid via resident Exp table (a Sigmoid activation would
            # force two ACT_TABLE_LOAD swaps on the scalar engine mid-kernel)
            e_neg = small.tile([BPC, 1], F32, tag="eneg")
            nc.scalar.activation(out=e_neg, in_=al_r, func=AF.Exp, bias=0.0, scale=-1.0)
            e_p1 = small.tile([BPC, 1], F32, tag="ep1")
            nc.vector.tensor_scalar_add(e_p1, e_neg, 1.0)
            sigv = small.tile([BPC, 1], F32, tag="sigv")
            nc.vector.reciprocal(sigv, e_p1)
            nc.scalar.mul(alb, sigv, -float(S) * INV_SG)  # alb = -aligned/sg
            # alignment roundtrip + per-batch broadcasts ride the otherwise
            # idle gpsimd ring so they never queue behind bulk traffic
            nc.gpsimd.dma_start(out=ab_d[:, :], in_=alb)
            for bb in range(BPC):
                nc.gpsimd.dma_start(
                    out=ab_tiles[bb], in_=ab_d[bb : bb + 1, :].to_broadcast((128, 1))
                )

        # ---- per-batch: scores + chunk-granular softmax/context ----
        # The bias m1 is fixed after chunk 3 (cols 0-7); every later chunk's
        # exp + 2 context matmuls run as soon as its score STT lands, so for
        # the last-streamed batch only chunk 7's sliver remains post-stream.
        # Exact softmax: e^{s-g2-m1}/sum(e^{s-m1}) == softmax(s)*gauss for
        # any bias m1 (worst |s-m1| here is far below fp32 exp overflow).
        NCH0 = 4          # chunks covered by the m1 bias phase
        C0 = NCH0 * A     # cols 0..7

        def batch_section(b, after_scores=None, mid_hook=None):
            ubc = ubc_tiles[b]
            sc_b = small.tile([128, COLS], F32, tag="scb", name=f"scb_{b}")

            def emit_stt(col):
                c, a = col // A, col % A
                prod = prodp.tile([128, H], F16, tag="p0", name=f"pr_{b}_{col}")
                nc.vector.scalar_tensor_tensor(
                    out=prod,
                    in0=all_x[b][c][:, a, :],
                    scalar=1.0,
                    in1=ubc,
                    op0=OP.mult,
                    op1=OP.mult,
                    accum_out=sc_b[:, col : col + 1],
                )

            for col in range(C0):
                emit_stt(col)

            # fix bias m1 = max over cols 0..7
            mx_p = small.tile([128, 1], F32, tag="mxp", name=f"mxp_{b}")
            nc.vector.reduce_max(
                out=mx_p, in_=sc_b[:, 0:C0], axis=mybir.AxisListType.X
            )
            mcast = small.tile([128, 1], F32, tag="mcast", name=f"mcast_{b}")
            nc.gpsimd.partition_all_reduce(
                mcast, mx_p, channels=128, reduce_op=bass_isa.ReduceOp.max
            )
            negm = small.tile([128, 1], F32, tag="negm", name=f"negm_{b}")
            nc.scalar.mul(negm, mcast, -1.0)

            # lookahead: chunk 4's score STTs go ahead of the g2-dependent
            # at0 multiply in the DVE queue (g2 can lag on batch 0)
            for col in range(C0, C0 + A):
                emit_stt(col)
            if after_scores is not None:
                after_scores()

            ew_all = small.tile([128, COLS], F32, tag="ew", name=f"ew_{b}")
            zp0 = small.tile([128, 1], F32, tag="zp0", name=f"zp0_{b}")
            nc.scalar.activation(
                out=ew_all[:, 0:C0],
                in_=sc_b[:, 0:C0],
                func=AF.Exp,
                bias=negm,
                scale=1.0,
                accum_out=zp0,
            )

            g2 = small.tile([128, COLS], F32, tag="g2", name=f"g2_{b}")
            nc.scalar.activation(
                out=g2, in_=pos_sb, func=AF.Square, bias=ab_tiles[b], scale=INV_SG
            )
            # gauss factors G = e^{-g2}
            gss = small.tile([128, COLS], F32, tag="gss", name=f"gss_{b}")
            nc.scalar.activation(out=gss, in_=g2, func=AF.Exp, bias=0.0, scale=-1.0)
            if mid_hook is not None:
                # PE work that's ready during this batch's softmax window
                mid_hook()

            # chunks 4..7: exp/Z accumulation fires per chunk as it lands;
            # the NEXT chunk's STTs are emitted first (lookahead) so no
            # scalar-dependent DVE op ever blocks a later score STT
            zrun = zp0
            for ch in range(NCH0, NCH):
                col0 = ch * A
                if ch + 1 < NCH:
                    for col in range((ch + 1) * A, (ch + 2) * A):
                        emit_stt(col)
                zpc = small.tile([128, 1], F32, tag=f"zp{ch}", name=f"zp{ch}_{b}")
                nc.scalar.activation(
                    out=ew_all[:, col0 : col0 + A],
                    in_=sc_b[:, col0 : col0 + A],
                    func=AF.Exp,
                    bias=negm,
                    scale=1.0,
                    accum_out=zpc,
                )
                zn = small.tile([128, 1], F32, tag=f"zr{ch}", name=f"zr{ch}_{b}")
                nc.vector.tensor_add(out=zn, in0=zrun, in1=zpc)
                zrun = zn

            zsum = small.tile([128, 1], F32, tag="zsum", name=f"zsum_{b}")
            nc.gpsimd.partition_all_reduce(
                zsum, zrun, channels=128, reduce_op=bass_isa.ReduceOp.add
            )
            zinv = small.tile([128, 1], F32, tag="zinv", name=f"zinv_{b}")
            nc.vector.reciprocal(zinv, zsum)

            # at16 = softmax * gauss = (e^{s-m1} * G) / Z, all <= 1 so it
            # fits fp16 (e^{s-m1} alone can reach e^38 and would overflow);
            # context is then one 32-matmul burst, pre-normalized
            at16 = small.tile([128, COLS], F16, tag="at16", name=f"at16_{b}")
            nc.vector.scalar_tensor_tensor(
                out=at16,
                in0=ew_all,
                scalar=zinv,
                in1=gss,
                op0=OP.mult,
                op1=OP.mult,
            )
            ps_c = psCtx.tile([1, H], F32, tag="pc", name=f"pc_{b}")
            for col in range(COLS):
                c, a = col // A, col % A
                for h2 in range(2):
                    nc.tensor.matmul(
                        ps_c[:, 512 * h2 : 512 * (h2 + 1)],
                        at16[:, col : col + 1],
                        all_x[b][c][:, a, 512 * h2 : 512 * (h2 + 1)],
                        start=(col == 0),
                        stop=(col == COLS - 1 and h2 == 1),
                    )

            # ctx out of PSUM (split across scalar+vector), transpose
            # 128-blocks, then one strided DVE write folds 1/Z and lands
            # all 8 combT columns for this batch
            ctx_t = ctxp.tile([1, H], F32, tag="ctx", name=f"ctx_{b}")
            nc.scalar.copy(ctx_t[0:1, 0 : H // 2], ps_c[0:1, 0 : H // 2])
            nc.vector.tensor_copy(out=ctx_t[0:1, H // 2 : H], in_=ps_c[0:1, H // 2 : H])
            ps_ct = psT.tile([128, 8], F32, tag="pt", name=f"pct_{b}")
            for k in range(8):
                nc.tensor.transpose(
                    ps_ct[:, k : k + 1],
                    ctx_t[0:1, 128 * k : 128 * (k + 1)],
                    id_sb[0:1, 0:1],
                )
            cT = combT[:, b : b + 1]
            comb_cols = bass.AP(
                tensor=cT.tensor, offset=cT.offset, ap=[cT.ap[0], [4, 8]]
            )
            nc.vector.tensor_copy(out=comb_cols, in_=ps_ct)

        # final-output accumulator [BPC, SIZE]; h_t-half matmuls run early
        ps_o = psO.tile([BPC, SIZE], F32, tag="po")

        def emit_final_hhalf():
            for k in range(8, 16):
                lhs = combT[:, 4 * k : 4 * k + 4]
                for h2 in range(2):
                    nc.tensor.matmul(
                        ps_o[:, 512 * h2 : 512 * (h2 + 1)],
                        lhs,
                        wv1_sb[:, k % 8, 512 * h2 : 512 * (h2 + 1)],
                        start=(k == 8),
                        stop=False,
                    )

        batch_section(0, after_scores=emit_aligned_section)
        emit_x_dmas(1, range(NCH))

        # W_v rows [H:2H] reuse W_aT's SBUF slot; DMA waits on ubc_3's reads
        # (all ubc broadcasts are emitted upfront, so that's ~25us in)
        wv1_sb = wts.tile([128, 8, SIZE], F16, tag="w1")
        nc.sync.dma_start(
            out=wv1_sb,
            in_=w_v[H : 2 * H, :].rearrange("(k p) o -> p k o", p=128),
        )
        batch_section(1)
        emit_x_dmas(2, range(NCH))
        batch_section(2, mid_hook=emit_final_hhalf)
        emit_x_dmas(3, range(NCH))

        # W_v rows [0:H] reuse W_p's slot; queued after all x so the x
        # stream is never stalled behind it (its matmuls are last anyway)
        wv0_sb = wts.tile([128, 8, SIZE], F16, tag="w0")
        for k in range(8):
            nc.sync.dma_start(
                out=wv0_sb[:, k : k + 1, :],
                in_=w_v[128 * k : 128 * (k + 1), :]
                .rearrange("(k p) o -> p k o", p=128),
            )
        batch_section(3)

        for k in range(8):
            lhs = combT[:, 4 * k : 4 * k + 4]
            for h2 in range(2):
                nc.tensor.matmul(
                    ps_o[:, 512 * h2 : 512 * (h2 + 1)],
                    lhs,
                    wv0_sb[:, k, 512 * h2 : 512 * (h2 + 1)],
                    start=False,
                    stop=(k == 7 and h2 == 1),
                )
        # tanh+store in quarters so each store overlaps the next tanh
        Q = SIZE // 4
        for q in range(4):
            nc.scalar.activation(
                out=out_sb[:, Q * q : Q * (q + 1)],
                in_=ps_o[:, Q * q : Q * (q + 1)],
                func=AF.Tanh,
            )
            ring = nc.gpsimd if q % 2 == 0 else nc.sync
            ring.dma_start(
                out=outd[:, Q * q : Q * (q + 1)], in_=out_sb[:, Q * q : Q * (q + 1)]
            )

    nc.compile()
    return nc


def _host_prep(x, W_p, v_p, W_a, W_v):
    x = np.asarray(x, dtype=np.float32)
    h_all = np.ascontiguousarray(x[:, -1, :])  # [B, H] exact fp32 h_t
    x = np.ascontiguousarray(x.astype(np.float16))
    W_p = np.ascontiguousarray(np.asarray(W_p, dtype=np.float16))
    v_p = np.asarray(v_p, dtype=np.float32).reshape(-1)
    W_aT = np.ascontiguousarray(np.asarray(W_a, dtype=np.float32).T.astype(np.float16))
    W_v = np.ascontiguousarray(np.asarray(W_v, dtype=np.float16))
    vrep = np.ascontiguousarray(np.broadcast_to(v_p.reshape(1, H), (BPC, H)))
    cols = np.arange(COLS)
    p = np.arange(128)
    pos = ((cols[None, :] // A) * SCH + p[:, None] * A + (cols[None, :] % A)).astype(
        np.float32
    )
    pos = np.ascontiguousarray(pos)
    ident = np.eye(128, dtype=np.float32)

    in_maps = []
    for c in range(NCORES):
        hT = h_all[BPC * c : BPC * (c + 1)].T.astype(np.float16)  # [H, BPC]
        htk_a = np.ascontiguousarray(
            hT.reshape(8, 128, BPC).transpose(1, 0, 2).reshape(128, 8 * BPC)
        )
        in_maps.append(
            dict(
                x_s=np.ascontiguousarray(x[BPC * c : BPC * (c + 1)]),
                w_p=W_p,
                w_at=W_aT,
                w_v=W_v,
                htk=htk_a,
                vrep=vrep,
                pos=pos,
                ident=ident,
            )
        )
    return in_maps


def kernel(x, W_p, v_p, W_a, W_v):
    if "nc" not in _CACHE:
        _CACHE["nc"] = _build()
    nc = _CACHE["nc"]
    in_maps = _host_prep(x, W_p, v_p, W_a, W_v)
    res = run_bass_kernel_spmd(nc, in_maps, core_ids=list(range(NCORES)), trace=TRACE)
    _CACHE["last_results"] = res
    return np.concatenate([r["out"] for r in res.results], axis=0)


# revision 37
# speedup vs baseline: 1.3050x; 1.1337x over previous
"""Trainium2 Bass kernel: Luong-style attention with predictive alignment.

Math (see reference):
    h_t    = x[:, -1, :]                                   [B, H]
    t      = tanh(h_t @ W_p);  aligned = S*sigmoid(t @ v_p)
    scores[b,s] = sum_h x[b,s,h] * u[b,h],  u[b] = W_a @ h_t[b]
        (algebraic rewrite of (x @ W_a) . h_t -- avoids the B*S*H*H einsum)
    attn   = softmax(scores) * exp(-(pos-aligned)^2 / sigma2)
    ctx[b] = sum_s attn[b,s] * x[b,s,:]
    out    = tanh(concat(ctx, h_t) @ W_v)

Sharding: data-parallel over batch. 8 cores x 4 batches each; weights
replicated per core.

Schedule (single priority-ordered DMA ring for all bulk traffic):
  W_aT | x b0 c0-3 | W_p | x b0 c4-7 | x b1 | W_v[H:] | x b2 | x b3 | W_v[:H]
W_v rows [H:2H] reuse W_aT's SBUF (after the last u-broadcast reads it) and
W_v rows [0:H] reuse W_p's; the W_v[:H] half streams after the last x tile so
x never waits, and its matmuls are the last accumulation group anyway.

Per-batch chunk-granular softmax: the bias m1 = max(scores[cols 0:8]) is
fixed after chunk 3; every later chunk's exp + 2 context matmuls fire as
soon as its score STT lands (with the next chunk's STTs emitted ahead of
any scalar-dependent DVE op, so the DVE FIFO never stalls on them).  After
the last x tile lands only chunk 7's sliver remains: 2 score STTs, one
small exp, 4 context matmuls, and the transpose/merge.  Exact softmax:
e^{s-g2-m1}/sum(e^{s-m1}) == softmax(s)*gauss for any bias m1 (worst
|s - m1| on this input distribution is ~38, far below fp32 exp overflow at
~88).  The gauss window is applied as a precomputed table G = e^{-g2}
multiplying e^{s-m1}, and the 1/Z normalization is folded into the single
strided DVE write that scatters the transposed context into combT's
per-batch columns.  sigmoid(z) is computed as 1/(1+e^{-z}) so the scalar
engine never swaps activation tables mid-kernel (Sigmoid lives in a
different table set than Exp/Square/Tanh; each swap costs ~1.5us).
"""

import math
from contextlib import ExitStack

import numpy as np

import concourse.bass as bass
import concourse.bass_isa as bass_isa
import concourse.mybir as mybir
import concourse.tile as tile
from concourse import bacc
from concourse.bass_utils import run_bass_kernel_spmd

B, S, H, SIZE = 32, 2048, 1024, 1024
NCORES = 8
BPC = B // NCORES          # batches per core
NCH = 8                    # x chunks per batch
SCH = S // NCH             # 256 sequence positions per chunk
A = 2                      # sub-slices (128 s-positions each) per chunk
COLS = NCH * A             # 16 score columns per batch
NCOLS_A = 14               # phase-A columns (chunks 0-6)
F32 = mybir.dt.float32
F32R = mybir.dt.float32r
F16 = mybir.dt.float16
SIGMA_SQ = 2.0 * (S / 2.0 / 2.0) ** 2    # D = S//2; 2*(D/2)^2 = 524288
INV_SG = 1.0 / math.sqrt(SIGMA_SQ)

_CACHE = {}
TRACE = False


def _build():
    AF = mybir.ActivationFunctionType
    OP = mybir.AluOpType
    nc = bacc.Bacc()

    x_s = nc.dram_tensor("x_s", [BPC, S, H], F16, kind="ExternalInput")
    w_p = nc.dram_tensor("w_p", [H, H], F16, kind="ExternalInput")
    w_at = nc.dram_tensor("w_at", [H, H], F16, kind="ExternalInput")
    w_v = nc.dram_tensor("w_v", [2 * H, SIZE], F16, kind="ExternalInput")
    htk = nc.dram_tensor("htk", [128, 8 * BPC], F16, kind="ExternalInput")
    vrep = nc.dram_tensor("vrep", [BPC, H], F32, kind="ExternalInput")
    posd = nc.dram_tensor("pos", [128, COLS], F32, kind="ExternalInput")
    idd = nc.dram_tensor("ident", [128, 128], F32, kind="ExternalInput")
    outd = nc.dram_tensor("out", [BPC, SIZE], F32, kind="ExternalOutput")

    with tile.TileContext(nc) as tc, ExitStack() as ctx:
        const = ctx.enter_context(tc.tile_pool(name="const", bufs=1))
        wts = ctx.enter_context(tc.tile_pool(name="wts", bufs=1))
        xs = ctx.enter_context(tc.tile_pool(name="xs", bufs=12))
        ubcp = ctx.enter_context(tc.tile_pool(name="ubcp", bufs=4))
        ctxp = ctx.enter_context(tc.tile_pool(name="ctxp", bufs=2))
        prodp = ctx.enter_context(tc.tile_pool(name="prodp", bufs=2))
        small = ctx.enter_context(tc.tile_pool(name="small", bufs=2))
        psUbc = ctx.enter_context(
            tc.tile_pool(name="psUbc", bufs=1, space=bass.MemorySpace.PSUM)
        )
        psCtx = ctx.enter_context(
            tc.tile_pool(name="psCtx", bufs=1, space=bass.MemorySpace.PSUM)
        )
        psT = ctx.enter_context(
            tc.tile_pool(name="psT", bufs=1, space=bass.MemorySpace.PSUM)
        )
        psO = ctx.enter_context(
            tc.tile_pool(name="psO", bufs=1, space=bass.MemorySpace.PSUM)
        )
        dpool = ctx.enter_context(
            tc.tile_pool(name="dram", bufs=1, space=bass.MemorySpace.DRAM)
        )

        # ---- small inputs ride the gpsimd ring; bulk traffic owns sync ----
        pad0 = const.tile([128, 320], F32)
        combT = const.tile([128, 8 * BPC * 2], F16)  # combined^T: [p, 4k+b]
        v_sb = const.tile([BPC, H], F32)
        pos_sb = const.tile([128, COLS], F32)
        id_sb = const.tile([128, 128], F32)
        tta = const.tile([BPC, H], F32)
        alb = const.tile([BPC, 1], F32)
        out_sb = const.tile([BPC, SIZE], F32)

        nc.gpsimd.dma_start(out=combT[:, 32:64], in_=htk[:, :])
        nc.gpsimd.dma_start(out=v_sb, in_=vrep[:, :])
        nc.gpsimd.dma_start(out=pos_sb, in_=posd[:, :])
        nc.gpsimd.dma_start(out=id_sb, in_=idd[:, :])

        # ---- bulk stream alternates between the sync and scalar queues so
        # one queue's descriptor-feed gaps are covered by the other; both
        # queues keep the same priority order: W_aT first (u gates scores)
        wa_sb = wts.tile([128, 8, H], F16, tag="w1")
        nc.sync.dma_start(
            out=wa_sb[:, 0:4, :],
            in_=w_at[0 : H // 2, :].rearrange("(k p) j -> p k j", p=128),
        )
        nc.sync.dma_start(
            out=wa_sb[:, 4:8, :],
            in_=w_at[H // 2 :, :].rearrange("(k p) j -> p k j", p=128),
        )

        # u[b] broadcast across partitions, computed directly on PE: lhsT is
        # the h_t column replicated along its free dim (step-0 AP), so
        # out[p, h] = sum_k h_t[b,k] W_aT[k,h] = u[b,h] for every partition p.
        ubc_tiles = [None] * BPC

        def emit_ubc(b):
            ub_ps = psUbc.tile([128, H], F32, tag="ub", name=f"ubps_{b}")
            for k in range(8):
                c0 = combT[:, 32 + 4 * k + b : 32 + 4 * k + b + 1]
                lhs = bass.AP(
                    tensor=c0.tensor, offset=c0.offset, ap=[c0.ap[0], [0, 128]]
                )
                for h2 in range(2):
                    nc.tensor.matmul(
                        ub_ps[:, 512 * h2 : 512 * (h2 + 1)],
                        lhs,
                        wa_sb[:, k, 512 * h2 : 512 * (h2 + 1)],
                        start=(k == 0),
                        stop=(k == 7),
                    )
            ubc = ubcp.tile([128, H], F16, tag="ubc", name=f"ubc_{b}")
            nc.scalar.copy(ubc, ub_ps)
            ubc_tiles[b] = ubc

        emit_ubc(0)
        emit_ubc(1)
        emit_ubc(2)
        emit_ubc(3)

        # ---- x DMAs (sync ring) ----
        all_x = [[None] * NCH for _ in range(BPC)]

        def emit_x_dmas(b, cs):
            for c in cs:
                xt = xs.tile([128, A, H], F16, tag="xt", name=f"xt_{b}_{c}")
                nc.sync.dma_start(
                    out=xt,
                    in_=x_s[b, c * SCH : (c + 1) * SCH, :]
                    .rearrange("(p a) h -> p a h", p=128),
                )
                all_x[b][c] = xt

        emit_x_dmas(0, range(4))

        # ---- W_p + t/aligned (f32r matmul; lhsT = combT h_t cols) ----
        wp_sb = wts.tile([128, 8, H], F16, tag="w0")
        nc.sync.dma_start(
            out=wp_sb, in_=w_p[:, :].rearrange("(k p) j -> p k j", p=128)
        )
        emit_x_dmas(0, range(4, NCH))
        ab_d = dpool.tile([BPC, 1], F32)
        ab_tiles = [
            const.tile([128, 1], F32, name=f"abb_{bb}") for bb in range(BPC)
        ]

        def emit_aligned_section():
            # t = tanh(h_t @ W_p); aligned = S*sigmoid(t @ v_p)
            ps_t = psO.tile([BPC, H], F32, tag="po")
            for k in range(8):
                lhs = combT[:, 32 + 4 * k : 32 + 4 * k + 4]
                for h2 in range(2):
                    nc.tensor.matmul(
                        ps_t[:, 512 * h2 : 512 * (h2 + 1)],
                        lhs,
                        wp_sb[:, k, 512 * h2 : 512 * (h2 + 1)],
                        start=(k == 0),
                        stop=(k == 7),
                    )
            nc.scalar.activation(out=tta, in_=ps_t, func=AF.Tanh)

            prod0 = prodp.tile([BPC, H], F32, tag="pal")
            al_r = small.tile([BPC, 1], F32, tag="alr")
            nc.vector.scalar_tensor_tensor(
                out=prod0,
                in0=tta,
                scalar=1.0,
                in1=v_sb,
                op0=OP.mult,
                op1=OP.mult,
                accum_out=al_r,
            )
            # sigmoid via resident Exp table (a Sigmoid activation would
            # force two ACT_TABLE_LOAD swaps on the scalar engine mid-kernel)
            e_neg = small.tile([BPC, 1], F32, tag="eneg")
            nc.scalar.activation(out=e_neg, in_=al_r, func=AF.Exp, bias=0.0, scale=-1.0)
            e_p1 = small.tile([BPC, 1], F32, tag="ep1")
            nc.vector.tensor_scalar_add(e_p1, e_neg, 1.0)
            sigv = small.tile([BPC, 1], F32, tag="sigv")
            nc.vector.reciprocal(sigv, e_p1)
            nc.scalar.mul(alb, sigv, -float(S) * INV_SG)  # alb = -aligned/sg
            # alignment roundtrip + per-batch broadcasts ride the otherwise
            # idle gpsimd ring so they never queue behind bulk traffic
            nc.gpsimd.dma_start(out=ab_d[:, :], in_=alb)
            for bb in range(BPC):
                nc.gpsimd.dma_start(
                    out=ab_tiles[bb], in_=ab_d[bb : bb + 1, :].to_broadcast((128, 1))
                )

        # ---- per-batch: scores + chunk-granular softmax/context ----
        # The bias m1 is fixed after chunk 3 (cols 0-7); every later chunk's
        # exp + 2 context matmuls run as soon as its score STT lands, so for
        # the last-streamed batch only chunk 7's sliver remains post-stream.
        # Exact softmax: e^{s-g2-m1}/sum(e^{s-m1}) == softmax(s)*gauss for
        # any bias m1 (worst |s-m1| here is far below fp32 exp overflow).
        NCH0 = 4          # chunks covered by the m1 bias phase
        C0 = NCH0 * A     # cols 0..7

        def batch_section(b, after_scores=None, mid_hook=None):
            ubc = ubc_tiles[b]
            sc_b = small.tile([128, COLS], F32, tag="scb", name=f"scb_{b}")

            def emit_stt(col):
                c, a = col // A, col % A
                prod = prodp.tile([128, H], F16, tag="p0", name=f"pr_{b}_{col}")
                nc.vector.scalar_tensor_tensor(
                    out=prod,
                    in0=all_x[b][c][:, a, :],
                    scalar=1.0,
                    in1=ubc,
                    op0=OP.mult,
                    op1=OP.mult,
                    accum_out=sc_b[:, col : col + 1],
                )

            for col in range(C0):
                emit_stt(col)

            # fix bias m1 = max over cols 0..7
            mx_p = small.tile([128, 1], F32, tag="mxp", name=f"mxp_{b}")
            nc.vector.reduce_max(
                out=mx_p, in_=sc_b[:, 0:C0], axis=mybir.AxisListType.X
            )
            mcast = small.tile([128, 1], F32, tag="mcast", name=f"mcast_{b}")
            nc.gpsimd.partition_all_reduce(
                mcast, mx_p, channels=128, reduce_op=bass_isa.ReduceOp.max
            )
            negm = small.tile([128, 1], F32, tag="negm", name=f"negm_{b}")
            nc.scalar.mul(negm, mcast, -1.0)

            # lookahead: chunk 4's score STTs go ahead of the g2-dependent
            # at0 multiply in the DVE queue (g2 can lag on batch 0)
            for col in range(C0, C0 + A):
                emit_stt(col)
            if after_scores is not None:
                after_scores()

            ew_all = small.tile([128, COLS], F32, tag="ew", name=f"ew_{b}")
            zp0 = small.tile([128, 1], F32, tag="zp0", name=f"zp0_{b}")
            nc.scalar.activation(
                out=ew_all[:, 0:C0],
                in_=sc_b[:, 0:C0],
                func=AF.Exp,
                bias=negm,
                scale=1.0,
                accum_out=zp0,
            )

            g2 = small.tile([128, COLS], F32, tag="g2", name=f"g2_{b}")
            nc.scalar.activation(
                out=g2, in_=pos_sb, func=AF.Square, bias=ab_tiles[b], scale=INV_SG
            )
            # gauss factors G = e^{-g2}
            gss = small.tile([128, COLS], F32, tag="gss", name=f"gss_{b}")
            nc.scalar.activation(out=gss, in_=g2, func=AF.Exp, bias=0.0, scale=-1.0)
            if mid_hook is not None:
                # PE work that's ready during this batch's softmax window
                mid_hook()

            # chunks 4..7: exp/Z accumulation fires per chunk as it lands;
            # the NEXT chunk's STTs are emitted first (lookahead) so no
            # scalar-dependent DVE op ever blocks a later score STT
            zrun = zp0
            for ch in range(NCH0, NCH):
                col0 = ch * A
                if ch + 1 < NCH:
                    for col in range((ch + 1) * A, (ch + 2) * A):
                        emit_stt(col)
                zpc = small.tile([128, 1], F32, tag=f"zp{ch}", name=f"zp{ch}_{b}")
                nc.scalar.activation(
                    out=ew_all[:, col0 : col0 + A],
                    in_=sc_b[:, col0 : col0 + A],
                    func=AF.Exp,
                    bias=negm,
                    scale=1.0,
                    accum_out=zpc,
                )
                zn = small.tile([128, 1], F32, tag=f"zr{ch}", name=f"zr{ch}_{b}")
                nc.vector.tensor_add(out=zn, in0=zrun, in1=zpc)
                zrun = zn

            zsum = small.tile([128, 1], F32, tag="zsum", name=f"zsum_{b}")
            nc.gpsimd.partition_all_reduce(
                zsum, zrun, channels=128, reduce_op=bass_isa.ReduceOp.add
            )
            zinv = small.tile([128, 1], F32, tag="zinv", name=f"zinv_{b}")
            nc.vector.reciprocal(zinv, zsum)

            # at16 = softmax * gauss = (e^{s-m1} * G) / Z, all <= 1 so it
            # fits fp16 (e^{s-m1} alone can reach e^38 and would overflow);
            # context is then one 32-matmul burst, pre-normalized
            at16 = small.tile([128, COLS], F16, tag="at16", name=f"at16_{b}")
            nc.vector.scalar_tensor_tensor(
                out=at16,
                in0=ew_all,
                scalar=zinv,
                in1=gss,
                op0=OP.mult,
                op1=OP.mult,
            )
            ps_c = psCtx.tile([1, H], F32, tag="pc", name=f"pc_{b}")
            for col in range(COLS):
                c, a = col // A, col % A
                for h2 in range(2):
                    nc.tensor.matmul(
                        ps_c[:, 512 * h2 : 512 * (h2 + 1)],
                        at16[:, col : col + 1],
                        all_x[b][c][:, a, 512 * h2 : 512 * (h2 + 1)],
                        start=(col == 0),
                        stop=(col == COLS - 1 and h2 == 1),
                    )

            # ctx out of PSUM (split across scalar+vector), transpose
            # 128-blocks, then one strided DVE write folds 1/Z and lands
            # all 8 combT columns for this batch
            ctx_t = ctxp.tile([1, H], F32, tag="ctx", name=f"ctx_{b}")
            nc.scalar.copy(ctx_t[0:1, 0 : H // 2], ps_c[0:1, 0 : H // 2])
            nc.vector.tensor_copy(out=ctx_t[0:1, H // 2 : H], in_=ps_c[0:1, H // 2 : H])
            ps_ct = psT.tile([128, 8], F32, tag="pt", name=f"pct_{b}")
            for k in range(8):
                nc.tensor.transpose(
                    ps_ct[:, k : k + 1],
                    ctx_t[0:1, 128 * k : 128 * (k + 1)],
                    id_sb[0:1, 0:1],
                )
            cT = combT[:, b : b + 1]
            comb_cols = bass.AP(
                tensor=cT.tensor, offset=cT.offset, ap=[cT.ap[0], [4, 8]]
            )
            nc.vector.tensor_copy(out=comb_cols, in_=ps_ct)

        # final-output accumulator [BPC, SIZE]; h_t-half matmuls run early
        ps_o = psO.tile([BPC, SIZE], F32, tag="po")

        def emit_final_hhalf():
            for k in range(8, 16):
                lhs = combT[:, 4 * k : 4 * k + 4]
                for h2 in range(2):
                    nc.tensor.matmul(
                        ps_o[:, 512 * h2 : 512 * (h2 + 1)],
                        lhs,
                        wv1_sb[:, k % 8, 512 * h2 : 512 * (h2 + 1)],
                        start=(k == 8),
                        stop=False,
                    )

        batch_section(0, after_scores=emit_aligned_section)
        emit_x_dmas(1, range(NCH))
        batch_section(1)
        emit_x_dmas(2, range(NCH))
        batch_section(2)
        emit_x_dmas(3, range(NCH))

        # both W_v halves stream AFTER all x (fp16 stream is short, so the
        # last batch's context burst is the tail; x must land first).  wv1
        # reuses W_aT's slot (ubc reads done ~27us), wv0 reuses W_p's.
        wv1_sb = wts.tile([128, 8, SIZE], F16, tag="w1")
        nc.sync.dma_start(
            out=wv1_sb,
            in_=w_v[H : 2 * H, :].rearrange("(k p) o -> p k o", p=128),
        )
        wv0_sb = wts.tile([128, 8, SIZE], F16, tag="w0")
        for k in range(8):
            nc.sync.dma_start(
                out=wv0_sb[:, k : k + 1, :],
                in_=w_v[128 * k : 128 * (k + 1), :]
                .rearrange("(k p) o -> p k o", p=128),
            )
        batch_section(3, mid_hook=emit_final_hhalf)

        for k in range(8):
            lhs = combT[:, 4 * k : 4 * k + 4]
            for h2 in range(2):
                nc.tensor.matmul(
                    ps_o[:, 512 * h2 : 512 * (h2 + 1)],
                    lhs,
                    wv0_sb[:, k, 512 * h2 : 512 * (h2 + 1)],
                    start=False,
                    stop=(k == 7 and h2 == 1),
                )
        # tanh+store in quarters so each store overlaps the next tanh
        Q = SIZE // 4
        for q in range(4):
            nc.scalar.activation(
                out=out_sb[:, Q * q : Q * (q + 1)],
                in_=ps_o[:, Q * q : Q * (q + 1)],
                func=AF.Tanh,
            )
            ring = nc.gpsimd if q % 2 == 0 else nc.sync
            ring.dma_start(
                out=outd[:, Q * q : Q * (q + 1)], in_=out_sb[:, Q * q : Q * (q + 1)]
            )

    nc.compile()
    return nc


def _host_prep(x, W_p, v_p, W_a, W_v):
    x = np.asarray(x, dtype=np.float32)
    h_all = np.ascontiguousarray(x[:, -1, :])  # [B, H] exact fp32 h_t
    x = np.ascontiguousarray(x.astype(np.float16))
    W_p = np.ascontiguousarray(np.asarray(W_p, dtype=np.float16))
    v_p = np.asarray(v_p, dtype=np.float32).reshape(-1)
    W_aT = np.ascontiguousarray(np.asarray(W_a, dtype=np.float32).T.astype(np.float16))
    W_v = np.ascontiguousarray(np.asarray(W_v, dtype=np.float16))
    vrep = np.ascontiguousarray(np.broadcast_to(v_p.reshape(1, H), (BPC, H)))
    cols = np.arange(COLS)
    p = np.arange(128)
    pos = ((cols[None, :] // A) * SCH + p[:, None] * A + (cols[None, :] % A)).astype(
        np.float32
    )
    pos = np.ascontiguousarray(pos)
    ident = np.eye(128, dtype=np.float32)

    in_maps = []
    for c in range(NCORES):
        hT = h_all[BPC * c : BPC * (c + 1)].T.astype(np.float16)  # [H, BPC]
        htk_a = np.ascontiguousarray(
            hT.reshape(8, 128, BPC).transpose(1, 0, 2).reshape(128, 8 * BPC)
        )
        in_maps.append(
            dict(
                x_s=np.ascontiguousarray(x[BPC * c : BPC * (c + 1)]),
                w_p=W_p,
                w_at=W_aT,
                w_v=W_v,
                htk=htk_a,
                vrep=vrep,
                pos=pos,
                ident=ident,
            )
        )
    return in_maps


def kernel(x, W_p, v_p, W_a, W_v):
    if "nc" not in _CACHE:
        _CACHE["nc"] = _build()
    nc = _CACHE["nc"]
    in_maps = _host_prep(x, W_p, v_p, W_a, W_v)
    res = run_bass_kernel_spmd(nc, in_maps, core_ids=list(range(NCORES)), trace=TRACE)
    _CACHE["last_results"] = res
    return np.concatenate([r["out"] for r in res.results], axis=0)
